# revision 8
# baseline (speedup 1.0000x reference)
"""Trainium2 Bass kernel for 3x3 same-padding Conv2d + bias (NCHW).

Problem: x[16,32,256,256] (*) weight[32,32,3,3] + bias[32] -> out[16,32,256,256]

Strategy (data-parallel over batch, 2 images per NeuronCore on 8 cores):
  - Host pre-shuffles x into an SBUF "slot" layout x_shuf[(g,ci)][s][b2][258]:
    image row h lives in row-group g=(h+1)%4 at slot s=(h+1)//4; the core's
    two batches are interleaved per slot; each row is 258 wide (zero pad col
    on each side) so the 3 horizontal conv taps are plain free-dim shifts.
    Device input DMAs are fully contiguous.
  - Output computed in "quads" of 4 consecutive rows for both batches at
    once: PSUM tiles [128, 2, 256] (full 2 KB banks) with partitions =
    (r, co).  Weight matrices (6 x [128,128], zero-padded per (g, r_out, kh)
    validity) are precomputed on the host from `weight`.
  - Default scheme ("f" variants) packs the PE as 4 concurrent 64x64 tiles
    (2x2 row/col tiling, tile_position auto-derived from base partitions):
    per kw tap, 4 quadrant matmuls cover the valid (g, r) blocks; ps1/ps2
    accumulate the two K-half contributions and are summed during PSUM
    evacuation (ScalarE copy + VectorE add).
  - "o" variants emit 16-bit outputs: the evacuation casts fp32 PSUM to
    fp16/bf16 so the store traffic halves; the host upcasts and adds bias.
  - Input loads ride the SP HWDGE ring; output stores ride the ACT HWDGE
    ring ("o" variants) so they do not share a FIFO with the loads.
"""
import sys

if "/opt/trn_rl_repo" not in sys.path:
    sys.path.insert(0, "/opt/trn_rl_repo")

import numpy as np

B, C, H, W = 16, 32, 256, 256
N_CORES = 8
PER = B // N_CORES          # batches per core (interleaved in the free dim)
HW = H * W
NSLOT = H // 4 + 1          # 65 row slots
SLOTW = W + 2               # 258 padded columns per slot
SLOTF = PER * SLOTW         # free-dim elements per slot (both batches)
NQ = H // 4                 # 64 quads
# progressive input sub-tile sizes (quads): small first so the PE starts
# within a few us, large later for DMA efficiency
QSIZES = [4, 6, 10, 14, 14, 16]
QSIZES_V3 = [2, 4, 8, 12, 12, 12, 14]   # all even: quad pairs never straddle

# dt_key -> config; pair=True processes quads in pairs (kw-major inside the
# pair so each PE tile runs 2 matmuls per weight load) and spreads DMA over
# both HWDGE rings + SWDGE.
VARIANTS = {
    "fp32r":  dict(in_dt="fp32r", packed=False, out_dt="fp32",
                   store="gpsimd", chunk=4, pair=False),
    "bf16":   dict(in_dt="bf16", packed=False, out_dt="fp32",
                   store="gpsimd", chunk=4, pair=False),
    "bf16f":  dict(in_dt="bf16", packed=True, out_dt="fp32",
                   store="gpsimd", chunk=4, pair=False),
    "bf16fo": dict(in_dt="bf16", packed=True, out_dt="bf16",
                   store="act", chunk=8, pair=False),
    "fp16fo": dict(in_dt="fp16", packed=True, out_dt="fp16",
                   store="act", chunk=8, pair=False),
    "fp16v3": dict(in_dt="fp16", packed=True, out_dt="fp16",
                   store="mixed", chunk=8, pair=True),
}

DT_KEY = "fp16v3"

_cache = {}


def _mybir_dt(name, mybir):
    return {"fp32r": mybir.dt.float32r, "fp32": mybir.dt.float32,
            "bf16": mybir.dt.bfloat16, "fp16": mybir.dt.float16}[name]


def _get_nc(dt_key=None):
    dt_key = dt_key or DT_KEY
    if dt_key in _cache:
        return _cache[dt_key]
    import concourse.mybir as mybir
    import concourse.tile as tile
    import concourse.bass as bass
    from concourse import bacc

    cfg = VARIANTS[dt_key]
    packed, ring, CHUNK, pair = (cfg["packed"], cfg["store"],
                                 cfg["chunk"], cfg["pair"])
    DT = _mybir_dt(cfg["in_dt"], mybir)
    OUT = _mybir_dt(cfg["out_dt"], mybir)
    F32 = mybir.dt.float32
    qsizes = QSIZES_V3 if pair else QSIZES

    nc = bacc.Bacc("TRN2", target_bir_lowering=False, debug=False,
                   num_devices=N_CORES)
    x_shuf = nc.dram_tensor("x_shuf", [128, NSLOT * SLOTF], DT,
                            kind="ExternalInput")
    w_taps = nc.dram_tensor("w_taps", [6, 128, 128], DT, kind="ExternalInput")
    out_shuf = nc.dram_tensor("out_shuf", [128, NQ * PER * W], OUT,
                              kind="ExternalOutput")

    assert sum(qsizes) == NQ
    with tile.TileContext(nc) as tc:
        with (
            tc.tile_pool(name="xin", bufs=1) as xpool,  # unique tag per tile
            tc.tile_pool(name="wts", bufs=1) as wpool,
            tc.tile_pool(name="stage", bufs=5) as spool,
            tc.tile_pool(name="psum", bufs=2 if pair else (4 if packed else 8),
                         space="PSUM") as ppool,
        ):
            # input sub-tiles with a 1-slot overlap, loaded as fully
            # contiguous DMAs.  pair mode: weights go first (tiny, unblocks
            # the HAM warm-up) and loads alternate between the two HWDGE
            # rings so their fixed costs overlap.
            w_t = wpool.tile([128, 6, 128], DT)
            if pair:
                nc.scalar.dma_start(
                    out=w_t[:], in_=w_taps.ap().rearrange("t k m -> k t m"))
            xts = []       # (tile, start_quad)
            q0 = 0
            for j, qsz in enumerate(qsizes):
                xt = xpool.tile([128, qsz + 1, PER, SLOTW], DT, tag=f"x{j}")
                lo = q0 * SLOTF
                hi = lo + (qsz + 1) * SLOTF
                src = (x_shuf.ap()[:, lo:hi]
                       .rearrange("p (s b w) -> p s b w", b=PER, w=SLOTW))
                if pair and j % 2 == 1:
                    nc.scalar.dma_start(out=xt[:], in_=src)
                else:
                    nc.sync.dma_start(out=xt[:], in_=src)
                xts.append((xt, q0))
                q0 += qsz
                if j == 0 and not pair:
                    nc.sync.dma_start(
                        out=w_t[:],
                        in_=w_taps.ap().rearrange("t k m -> k t m"))
            starts = np.cumsum([0] + qsizes).tolist()

            # HAM warm-up: dummy matmuls during the first load so the PE
            # clock is already at 2.4 GHz when real work arrives (N=256 so
            # float32r runs at full rate too)
            wm = ppool.tile([128, PER, W], F32,
                            tag="psa1" if pair else ("ps1" if packed else "ps"))
            for _ in range(16 if pair else 10):
                nc.tensor.matmul(wm[0:64, 0, :], w_t[0:64, 0, 0:64],
                                 w_t[0:64, 0:2, :], start=True, stop=True)

            def _locate(u):
                j = next(i for i in range(len(qsizes)) if starts[i + 1] > u)
                xt, tq0 = xts[j]
                return xt, u - tq0

            def _mm4(ps1, ps2, xt, lu, kw):
                """One kw tap of quad at slot lu: 4 concurrent 64x64 PE
                tiles; ps1/ps2 hold the two K-half contributions."""
                s0, s1 = (kw == 0), (kw == 2)
                nc.tensor.matmul(ps1[0:64, :, :], w_t[0:64, kw * 2, 0:64],
                                 xt[0:64, lu, :, kw:kw + W],
                                 start=s0, stop=s1)
                nc.tensor.matmul(ps2[0:64, :, :], w_t[64:128, kw * 2, 0:64],
                                 xt[64:128, lu, :, kw:kw + W],
                                 start=s0, stop=s1)
                nc.tensor.matmul(ps2[64:128, :, :],
                                 w_t[64:128, kw * 2, 64:128],
                                 xt[64:128, lu, :, kw:kw + W],
                                 start=s0, stop=s1)
                nc.tensor.matmul(ps1[64:128, :, :],
                                 w_t[0:64, kw * 2 + 1, 64:128],
                                 xt[0:64, lu + 1, :, kw:kw + W],
                                 start=s0, stop=s1)

            def _evac(p1, p2, st, ql):
                tmp = spool.tile([128, PER, W], F32, tag="ps2tmp")
                nc.scalar.copy(tmp[:], p2[:])
                nc.vector.tensor_add(st[:, ql, :, :], p1[:], tmp[:])

            for k in range(NQ // CHUNK):
                st = spool.tile([128, CHUNK, PER, W], OUT)
                if pair:
                    for qp in range(CHUNK // 2):
                        u = k * CHUNK + qp * 2
                        xt, lu = _locate(u)   # u+1 in same tile (even sizes)
                        psa1 = ppool.tile([128, PER, W], F32)
                        psa2 = ppool.tile([128, PER, W], F32)
                        psb1 = ppool.tile([128, PER, W], F32)
                        psb2 = ppool.tile([128, PER, W], F32)
                        # kw-major over the pair: each PE tile issues its two
                        # quads' matmuls back-to-back on one stationary
                        for kw in range(3):
                            s0, s1 = (kw == 0), (kw == 2)
                            for ps, l in ((psa1, lu), (psb1, lu + 1)):
                                nc.tensor.matmul(
                                    ps[0:64, :, :], w_t[0:64, kw * 2, 0:64],
                                    xt[0:64, l, :, kw:kw + W],
                                    start=s0, stop=s1)
                            for ps, l in ((psa2, lu), (psb2, lu + 1)):
                                nc.tensor.matmul(
                                    ps[0:64, :, :], w_t[64:128, kw * 2, 0:64],
                                    xt[64:128, l, :, kw:kw + W],
                                    start=s0, stop=s1)
                            for ps, l in ((psa2, lu), (psb2, lu + 1)):
                                nc.tensor.matmul(
                                    ps[64:128, :, :],
                                    w_t[64:128, kw * 2, 64:128],
                                    xt[64:128, l, :, kw:kw + W],
                                    start=s0, stop=s1)
                            for ps, l in ((psa1, lu + 1), (psb1, lu + 2)):
                                nc.tensor.matmul(
                                    ps[64:128, :, :],
                                    w_t[0:64, kw * 2 + 1, 64:128],
                                    xt[0:64, l, :, kw:kw + W],
                                    start=s0, stop=s1)
                        _evac(psa1, psa2, st, qp * 2)
                        _evac(psb1, psb2, st, qp * 2 + 1)
                else:
                    for ql in range(CHUNK):
                        u = k * CHUNK + ql
                        xt, lu = _locate(u)
                        if not packed:
                            ps = ppool.tile([128, PER, W], F32)
                            for kw in range(3):
                                nc.tensor.matmul(ps[:], w_t[:, kw * 2, :],
                                                 xt[:, lu, :, kw:kw + W],
                                                 start=(kw == 0), stop=False)
                                nc.tensor.matmul(ps[:], w_t[:, kw * 2 + 1, :],
                                                 xt[:, lu + 1, :, kw:kw + W],
                                                 start=False, stop=(kw == 2))
                            if ql % 2 == 0:
                                nc.vector.tensor_copy(st[:, ql, :, :], ps[:])
                            else:
                                nc.scalar.copy(st[:, ql, :, :], ps[:])
                        else:
                            ps1 = ppool.tile([128, PER, W], F32)
                            ps2 = ppool.tile([128, PER, W], F32)
                            for kw in range(3):
                                _mm4(ps1, ps2, xt, lu, kw)
                            _evac(ps1, ps2, st, ql)
                # contiguous per-partition store of CHUNK quads
                dst = bass.AP(out_shuf, k * CHUNK * PER * W,
                              [[NQ * PER * W, 128], [1, CHUNK * PER * W]])
                src = st[:].rearrange("p q b w -> p (q b w)")
                nchunk = NQ // CHUNK
                if ring == "act":
                    nc.scalar.dma_start(out=dst, in_=src)
                elif ring == "mixed":
                    # bulk on SWDGE; final two chunks on the (by then idle)
                    # HWDGE rings so the tail isn't queued behind anything
                    if k == nchunk - 2:
                        nc.sync.dma_start(out=dst, in_=src)
                    elif k == nchunk - 1:
                        nc.scalar.dma_start(out=dst, in_=src)
                    else:
                        nc.gpsimd.dma_start(out=dst, in_=src)
                else:
                    nc.gpsimd.dma_start(out=dst, in_=src)

    nc.compile()
    _cache[dt_key] = nc
    return nc


def _make_w_taps(weight):
    """Zero-padded stationary matrices w_taps[kw*2+part][(g,ci), (r,co)]."""
    w_taps = np.zeros((6, 128, 128), dtype=np.float32)
    for kw in range(3):
        for g in range(4):
            for r in range(4):
                kh0 = g - r              # window W_u (input row 4u+g-1)
                if 0 <= kh0 <= 2:
                    w_taps[kw * 2, g * 32:(g + 1) * 32, r * 32:(r + 1) * 32] = \
                        weight[:, :, kh0, kw].T
                kh1 = g - r + 4          # window W_{u+1} (input row 4u+g+3)
                if 0 <= kh1 <= 2:
                    w_taps[kw * 2 + 1, g * 32:(g + 1) * 32, r * 32:(r + 1) * 32] = \
                        weight[:, :, kh1, kw].T
    return w_taps


def _np_dt(dt_key):
    in_dt = VARIANTS[dt_key]["in_dt"]
    if in_dt == "fp16":
        return np.float16
    if in_dt == "bf16":
        import ml_dtypes
        return ml_dtypes.bfloat16
    return np.float32


def _shuffle_x(x, np_dt=np.float32):
    """x[B,C,H,W] -> per-core x_shuf[N_CORES,128,NSLOT,PER,SLOTW]: row h ->
    (group (h+1)%4, slot (h+1)//4), batch pair interleaved, cols 1..W data,
    zero pads elsewhere."""
    xc = x.reshape(N_CORES, PER, C, H, W)
    xs = np.zeros((N_CORES, 128, NSLOT, PER, SLOTW), dtype=np_dt)
    # group g, slot s holds row 4s+g-1
    xs[:, 0:32, 1:NSLOT, :, 1:W + 1] = \
        xc[:, :, :, 3::4, :].transpose(0, 2, 3, 1, 4).astype(np_dt)
    xs[:, 32:64, 0:NSLOT - 1, :, 1:W + 1] = \
        xc[:, :, :, 0::4, :].transpose(0, 2, 3, 1, 4).astype(np_dt)
    xs[:, 64:96, 0:NSLOT - 1, :, 1:W + 1] = \
        xc[:, :, :, 1::4, :].transpose(0, 2, 3, 1, 4).astype(np_dt)
    xs[:, 96:128, 0:NSLOT - 1, :, 1:W + 1] = \
        xc[:, :, :, 2::4, :].transpose(0, 2, 3, 1, 4).astype(np_dt)
    return xs.reshape(N_CORES, 128, NSLOT * SLOTF)


def _unshuffle_out(chunks):
    """chunks: per-core [128, NQ*PER*W] -> out[B,C,H,W] (fp32)."""
    o = np.stack(chunks, axis=0).astype(np.float32)   # [8, 128, NQ*PER*W]
    o = o.reshape(N_CORES, 4, C, NQ, PER, W)      # [c, r, co, q, b2, w]
    o = o.transpose(0, 4, 2, 3, 1, 5)             # [c, b2, co, q, r, w]
    return np.ascontiguousarray(o.reshape(B, C, H, W))


def make_in_maps(x, weight, dt_key=None):
    dt_key = dt_key or DT_KEY
    np_dt = _np_dt(dt_key)
    w_taps = _make_w_taps(np.asarray(weight, dtype=np.float32)).astype(np_dt)
    x_shuf = _shuffle_x(np.asarray(x, dtype=np.float32), np_dt)
    return [{"x_shuf": x_shuf[c], "w_taps": w_taps} for c in range(N_CORES)]


def kernel(x, weight, bias):
    from concourse.bass_utils import run_bass_kernel_spmd

    bias = np.asarray(bias, dtype=np.float32)
    nc = _get_nc()
    in_maps = make_in_maps(x, weight)
    res = run_bass_kernel_spmd(nc, in_maps, list(range(N_CORES)))
    out = _unshuffle_out([res.results[c]["out_shuf"] for c in range(N_CORES)])
    out += bias.reshape(1, C, 1, 1)
    return out


# revision 16
# speedup vs baseline: 1.1808x; 1.1808x over previous
"""Trainium2 Bass kernel for 3x3 same-padding Conv2d + bias (NCHW).

Problem: x[16,32,256,256] (*) weight[32,32,3,3] + bias[32] -> out[16,32,256,256]

Strategy (data-parallel over batch, 2 images per NeuronCore on 8 cores):
  - Host pre-shuffles x into an SBUF "slot" layout x_shuf[(g,ci)][s][b2][258]:
    image row h lives in row-group g=(h+1)%4 at slot s=(h+1)//4; the core's
    two batches are interleaved per slot; each row is 258 wide (zero pad col
    on each side) so the 3 horizontal conv taps are plain free-dim shifts.
    Device input DMAs are fully contiguous.
  - Output computed in "quads" of 4 consecutive rows for both batches at
    once: PSUM tiles [128, 2, 256] (full 2 KB banks) with partitions =
    (r, co).  Weight matrices (6 x [128,128], zero-padded per (g, r_out, kh)
    validity) are precomputed on the host from `weight`.
  - Default scheme ("f" variants) packs the PE as 4 concurrent 64x64 tiles
    (2x2 row/col tiling, tile_position auto-derived from base partitions):
    per kw tap, 4 quadrant matmuls cover the valid (g, r) blocks; ps1/ps2
    accumulate the two K-half contributions and are summed during PSUM
    evacuation (ScalarE copy + VectorE add).
  - "o" variants emit 16-bit outputs: the evacuation casts fp32 PSUM to
    fp16/bf16 so the store traffic halves; the host upcasts and adds bias.
  - Input loads ride the SP HWDGE ring; output stores ride the ACT HWDGE
    ring ("o" variants) so they do not share a FIFO with the loads.
"""
import sys

if "/opt/trn_rl_repo" not in sys.path:
    sys.path.insert(0, "/opt/trn_rl_repo")

import numpy as np

B, C, H, W = 16, 32, 256, 256
N_CORES = 8
PER = B // N_CORES          # batches per core (interleaved in the free dim)
HW = H * W
NSLOT = H // 4 + 1          # 65 row slots
SLOTW = W + 2               # 258 padded columns per slot
SLOTF = PER * SLOTW         # free-dim elements per slot (both batches)
NQ = H // 4                 # 64 quads
# progressive input sub-tile sizes (quads): small first so the PE starts
# within a few us, large later for DMA efficiency
QSIZES = [4, 6, 10, 14, 14, 16]
QSIZES_V3 = [2, 4, 8, 12, 12, 12, 14]   # all even: quad pairs never straddle

# dt_key -> config; pair=True processes quads in pairs (kw-major inside the
# pair so each PE tile runs 2 matmuls per weight load) and spreads DMA over
# both HWDGE rings + SWDGE.
VARIANTS = {
    "fp32r":  dict(in_dt="fp32r", packed=False, out_dt="fp32",
                   store="gpsimd", chunk=4, pair=False),
    "bf16":   dict(in_dt="bf16", packed=False, out_dt="fp32",
                   store="gpsimd", chunk=4, pair=False),
    "bf16f":  dict(in_dt="bf16", packed=True, out_dt="fp32",
                   store="gpsimd", chunk=4, pair=False),
    "bf16fo": dict(in_dt="bf16", packed=True, out_dt="bf16",
                   store="act", chunk=8, pair=False),
    "fp16fo": dict(in_dt="fp16", packed=True, out_dt="fp16",
                   store="act", chunk=8, pair=False),
    "fp16v3": dict(in_dt="fp16", packed=True, out_dt="fp16",
                   store="mixed", chunk=8, pair=True),
    "fp16v4": dict(in_dt="fp16", packed=True, out_dt="fp16",
                   store="v4", chunk=8, pair=False, fast=True),
    "fp16v5": dict(in_dt="fp16", packed=True, out_dt="fp16",
                   store="v4", chunk=8, pair=False, fast=True, tmp16=True),
}

DT_KEY = "fp16v4"

_cache = {}


def _mybir_dt(name, mybir):
    return {"fp32r": mybir.dt.float32r, "fp32": mybir.dt.float32,
            "bf16": mybir.dt.bfloat16, "fp16": mybir.dt.float16}[name]


def _get_nc(dt_key=None):
    dt_key = dt_key or DT_KEY
    if dt_key in _cache:
        return _cache[dt_key]
    import concourse.mybir as mybir
    import concourse.tile as tile
    import concourse.bass as bass
    from concourse import bacc

    cfg = VARIANTS[dt_key]
    packed, ring, CHUNK, pair = (cfg["packed"], cfg["store"],
                                 cfg["chunk"], cfg["pair"])
    fast = cfg.get("fast", False)
    tmp16 = cfg.get("tmp16", False)
    DT = _mybir_dt(cfg["in_dt"], mybir)
    OUT = _mybir_dt(cfg["out_dt"], mybir)
    F32 = mybir.dt.float32
    qsizes = QSIZES_V3 if (pair or fast) else QSIZES

    nc = bacc.Bacc("TRN2", target_bir_lowering=False, debug=False,
                   num_devices=N_CORES)
    x_shuf = nc.dram_tensor("x_shuf", [128, NSLOT * SLOTF], DT,
                            kind="ExternalInput")
    w_taps = nc.dram_tensor("w_taps", [6, 128, 128], DT, kind="ExternalInput")
    out_shuf = nc.dram_tensor("out_shuf", [128, NQ * PER * W], OUT,
                              kind="ExternalOutput")

    assert sum(qsizes) == NQ
    with tile.TileContext(nc) as tc:
        with (
            tc.tile_pool(name="xin", bufs=1) as xpool,  # unique tag per tile
            tc.tile_pool(name="wts", bufs=1) as wpool,
            tc.tile_pool(name="stage", bufs=5) as spool,
            tc.tile_pool(name="psum", bufs=2 if pair else (4 if packed else 8),
                         space="PSUM") as ppool,
        ):
            # input sub-tiles with a 1-slot overlap, loaded as fully
            # contiguous DMAs.  fast mode: weight load rides SWDGE (own Q7
            # path), odd x tiles ride the ACT HWDGE ring, even tiles the SP
            # ring, so the serial per-ring drain time halves; the first
            # loads are pinned to the front via high_priority.
            w_t = wpool.tile([128, 6, 128], DT)
            w_src = w_taps.ap().rearrange("t k m -> k t m")
            if fast:
                with tc.high_priority():
                    nc.gpsimd.dma_start(out=w_t[:], in_=w_src)
            elif pair:
                nc.scalar.dma_start(out=w_t[:], in_=w_src)
            xts = []       # (tile, start_quad)
            q0 = 0
            for j, qsz in enumerate(qsizes):
                xt = xpool.tile([128, qsz + 1, PER, SLOTW], DT, tag=f"x{j}")
                lo = q0 * SLOTF
                hi = lo + (qsz + 1) * SLOTF
                src = (x_shuf.ap()[:, lo:hi]
                       .rearrange("p (s b w) -> p s b w", b=PER, w=SLOTW))
                if fast:
                    eng = nc.scalar if j % 2 == 1 else nc.sync
                    if j <= 1:
                        with tc.high_priority():
                            eng.dma_start(out=xt[:], in_=src)
                    else:
                        eng.dma_start(out=xt[:], in_=src)
                elif pair and j % 2 == 1:
                    nc.scalar.dma_start(out=xt[:], in_=src)
                else:
                    nc.sync.dma_start(out=xt[:], in_=src)
                xts.append((xt, q0))
                q0 += qsz
                if j == 0 and not (pair or fast):
                    nc.sync.dma_start(
                        out=w_t[:],
                        in_=w_taps.ap().rearrange("t k m -> k t m"))
            starts = np.cumsum([0] + qsizes).tolist()

            # HAM warm-up: dummy matmuls so the PE clock is already at
            # 2.4 GHz when real work arrives.  fast mode feeds a memset
            # SBUF tile so the warm-up has no DMA dependency and ramps
            # from engine boot.
            wm = ppool.tile([128, PER, W], F32,
                            tag="psa1" if pair else ("ps1" if packed else "ps"))
            if fast:
                dummy = wpool.tile([128, 256], DT, tag="dummy")
                with tc.high_priority():
                    nc.vector.memset(dummy[0:64, :], 1.0)
                    for _ in range(16):
                        nc.tensor.matmul(wm[0:64, 0, :], dummy[0:64, 0:64],
                                         dummy[0:64, :], start=True, stop=True)
            else:
                for _ in range(16 if pair else 10):
                    nc.tensor.matmul(wm[0:64, 0, :], w_t[0:64, 0, 0:64],
                                     w_t[0:64, 0:2, :], start=True, stop=True)

            def _locate(u):
                j = next(i for i in range(len(qsizes)) if starts[i + 1] > u)
                xt, tq0 = xts[j]
                return xt, u - tq0

            def _mm4(ps1, ps2, xt, lu, kw):
                """One kw tap of quad at slot lu: 4 concurrent 64x64 PE
                tiles; ps1/ps2 hold the two K-half contributions."""
                s0, s1 = (kw == 0), (kw == 2)
                nc.tensor.matmul(ps1[0:64, :, :], w_t[0:64, kw * 2, 0:64],
                                 xt[0:64, lu, :, kw:kw + W],
                                 start=s0, stop=s1)
                nc.tensor.matmul(ps2[0:64, :, :], w_t[64:128, kw * 2, 0:64],
                                 xt[64:128, lu, :, kw:kw + W],
                                 start=s0, stop=s1)
                nc.tensor.matmul(ps2[64:128, :, :],
                                 w_t[64:128, kw * 2, 64:128],
                                 xt[64:128, lu, :, kw:kw + W],
                                 start=s0, stop=s1)
                nc.tensor.matmul(ps1[64:128, :, :],
                                 w_t[0:64, kw * 2 + 1, 64:128],
                                 xt[0:64, lu + 1, :, kw:kw + W],
                                 start=s0, stop=s1)

            def _evac(p1, p2, st, ql):
                tmp = spool.tile([128, PER, W], OUT if tmp16 else F32,
                                 tag="ps2tmp")
                nc.scalar.copy(tmp[:], p2[:])
                nc.vector.tensor_add(st[:, ql, :, :], p1[:], tmp[:])

            if ring == "v4":
                # final stores split small across both HWDGE rings so the
                # tail after the last evacuation is one short transfer
                chunk_plan = [CHUNK] * (NQ // CHUNK - 1) + [CHUNK // 2] * 2
            else:
                chunk_plan = [CHUNK] * (NQ // CHUNK)
            qoff = 0
            for k, CH in enumerate(chunk_plan):
                st = spool.tile([128, CHUNK, PER, W], OUT)
                if pair:
                    for qp in range(CH // 2):
                        u = qoff + qp * 2
                        xt, lu = _locate(u)   # u+1 in same tile (even sizes)
                        psa1 = ppool.tile([128, PER, W], F32)
                        psa2 = ppool.tile([128, PER, W], F32)
                        psb1 = ppool.tile([128, PER, W], F32)
                        psb2 = ppool.tile([128, PER, W], F32)
                        # kw-major over the pair: each PE tile issues its two
                        # quads' matmuls back-to-back on one stationary
                        for kw in range(3):
                            s0, s1 = (kw == 0), (kw == 2)
                            for ps, l in ((psa1, lu), (psb1, lu + 1)):
                                nc.tensor.matmul(
                                    ps[0:64, :, :], w_t[0:64, kw * 2, 0:64],
                                    xt[0:64, l, :, kw:kw + W],
                                    start=s0, stop=s1)
                            for ps, l in ((psa2, lu), (psb2, lu + 1)):
                                nc.tensor.matmul(
                                    ps[0:64, :, :], w_t[64:128, kw * 2, 0:64],
                                    xt[64:128, l, :, kw:kw + W],
                                    start=s0, stop=s1)
                            for ps, l in ((psa2, lu), (psb2, lu + 1)):
                                nc.tensor.matmul(
                                    ps[64:128, :, :],
                                    w_t[64:128, kw * 2, 64:128],
                                    xt[64:128, l, :, kw:kw + W],
                                    start=s0, stop=s1)
                            for ps, l in ((psa1, lu + 1), (psb1, lu + 2)):
                                nc.tensor.matmul(
                                    ps[64:128, :, :],
                                    w_t[0:64, kw * 2 + 1, 64:128],
                                    xt[0:64, l, :, kw:kw + W],
                                    start=s0, stop=s1)
                        _evac(psa1, psa2, st, qp * 2)
                        _evac(psb1, psb2, st, qp * 2 + 1)
                else:
                    for ql in range(CH):
                        u = qoff + ql
                        xt, lu = _locate(u)
                        if not packed:
                            ps = ppool.tile([128, PER, W], F32)
                            for kw in range(3):
                                nc.tensor.matmul(ps[:], w_t[:, kw * 2, :],
                                                 xt[:, lu, :, kw:kw + W],
                                                 start=(kw == 0), stop=False)
                                nc.tensor.matmul(ps[:], w_t[:, kw * 2 + 1, :],
                                                 xt[:, lu + 1, :, kw:kw + W],
                                                 start=False, stop=(kw == 2))
                            if ql % 2 == 0:
                                nc.vector.tensor_copy(st[:, ql, :, :], ps[:])
                            else:
                                nc.scalar.copy(st[:, ql, :, :], ps[:])
                        else:
                            ps1 = ppool.tile([128, PER, W], F32)
                            ps2 = ppool.tile([128, PER, W], F32)
                            for kw in range(3):
                                _mm4(ps1, ps2, xt, lu, kw)
                            _evac(ps1, ps2, st, ql)
                # contiguous per-partition store of CH quads
                dst = bass.AP(out_shuf, qoff * PER * W,
                              [[NQ * PER * W, 128], [1, CH * PER * W]])
                src = st[:, 0:CH].rearrange("p q b w -> p (q b w)")
                nchunk = len(chunk_plan)
                if ring == "act":
                    nc.scalar.dma_start(out=dst, in_=src)
                elif ring == "v4":
                    # bulk on the ACT ring; the two small final chunks go to
                    # the two HWDGE rings so they drain concurrently
                    if k == nchunk - 2:
                        nc.sync.dma_start(out=dst, in_=src)
                    else:
                        nc.scalar.dma_start(out=dst, in_=src)
                elif ring == "mixed":
                    if k == nchunk - 2:
                        nc.sync.dma_start(out=dst, in_=src)
                    elif k == nchunk - 1:
                        nc.scalar.dma_start(out=dst, in_=src)
                    else:
                        nc.gpsimd.dma_start(out=dst, in_=src)
                else:
                    nc.gpsimd.dma_start(out=dst, in_=src)
                qoff += CH

    nc.compile()
    _cache[dt_key] = nc
    return nc


def _make_w_taps(weight):
    """Zero-padded stationary matrices w_taps[kw*2+part][(g,ci), (r,co)]."""
    w_taps = np.zeros((6, 128, 128), dtype=np.float32)
    for kw in range(3):
        for g in range(4):
            for r in range(4):
                kh0 = g - r              # window W_u (input row 4u+g-1)
                if 0 <= kh0 <= 2:
                    w_taps[kw * 2, g * 32:(g + 1) * 32, r * 32:(r + 1) * 32] = \
                        weight[:, :, kh0, kw].T
                kh1 = g - r + 4          # window W_{u+1} (input row 4u+g+3)
                if 0 <= kh1 <= 2:
                    w_taps[kw * 2 + 1, g * 32:(g + 1) * 32, r * 32:(r + 1) * 32] = \
                        weight[:, :, kh1, kw].T
    return w_taps


def _np_dt(dt_key):
    in_dt = VARIANTS[dt_key]["in_dt"]
    if in_dt == "fp16":
        return np.float16
    if in_dt == "bf16":
        import ml_dtypes
        return ml_dtypes.bfloat16
    return np.float32


def _shuffle_x(x, np_dt=np.float32):
    """x[B,C,H,W] -> per-core x_shuf[N_CORES,128,NSLOT,PER,SLOTW]: row h ->
    (group (h+1)%4, slot (h+1)//4), batch pair interleaved, cols 1..W data,
    zero pads elsewhere."""
    xc = x.reshape(N_CORES, PER, C, H, W)
    xs = np.zeros((N_CORES, 128, NSLOT, PER, SLOTW), dtype=np_dt)
    # group g, slot s holds row 4s+g-1
    xs[:, 0:32, 1:NSLOT, :, 1:W + 1] = \
        xc[:, :, :, 3::4, :].transpose(0, 2, 3, 1, 4).astype(np_dt)
    xs[:, 32:64, 0:NSLOT - 1, :, 1:W + 1] = \
        xc[:, :, :, 0::4, :].transpose(0, 2, 3, 1, 4).astype(np_dt)
    xs[:, 64:96, 0:NSLOT - 1, :, 1:W + 1] = \
        xc[:, :, :, 1::4, :].transpose(0, 2, 3, 1, 4).astype(np_dt)
    xs[:, 96:128, 0:NSLOT - 1, :, 1:W + 1] = \
        xc[:, :, :, 2::4, :].transpose(0, 2, 3, 1, 4).astype(np_dt)
    return xs.reshape(N_CORES, 128, NSLOT * SLOTF)


def _unshuffle_out(chunks):
    """chunks: per-core [128, NQ*PER*W] -> out[B,C,H,W] (fp32)."""
    o = np.stack(chunks, axis=0).astype(np.float32)   # [8, 128, NQ*PER*W]
    o = o.reshape(N_CORES, 4, C, NQ, PER, W)      # [c, r, co, q, b2, w]
    o = o.transpose(0, 4, 2, 3, 1, 5)             # [c, b2, co, q, r, w]
    return np.ascontiguousarray(o.reshape(B, C, H, W))


def make_in_maps(x, weight, dt_key=None):
    dt_key = dt_key or DT_KEY
    np_dt = _np_dt(dt_key)
    w_taps = _make_w_taps(np.asarray(weight, dtype=np.float32)).astype(np_dt)
    x_shuf = _shuffle_x(np.asarray(x, dtype=np.float32), np_dt)
    return [{"x_shuf": x_shuf[c], "w_taps": w_taps} for c in range(N_CORES)]


def kernel(x, weight, bias):
    from concourse.bass_utils import run_bass_kernel_spmd

    bias = np.asarray(bias, dtype=np.float32)
    nc = _get_nc()
    in_maps = make_in_maps(x, weight)
    res = run_bass_kernel_spmd(nc, in_maps, list(range(N_CORES)))
    out = _unshuffle_out([res.results[c]["out_shuf"] for c in range(N_CORES)])
    out += bias.reshape(1, C, 1, 1)
    return out
